# revision 1
# baseline (speedup 1.0000x reference)
"""GCN EndNodeSelector Bass kernel for TRN2, 8-core SPMD.

Pipeline (per core, nodes row-sharded, degree-sorted within core):
  P1: z1 = x @ W1 (PE, xT streamed from DRAM), z1' = dinv * z1
  P2: AllGather z1' -> z1g (per-core chunks of NPC+1 rows; last row zero)
  P3: conv1 aggregation via dma_gather (LO/HI windows) + strided
      tensor_reduce; h1 = dinv * sum + b1 ; u = elu(h1)
  P4: AllGather u -> ug
  P5: x1 gather from ug (mapping winners), hcat=[u|x1u], z2 = hcat @ W2
  P6: AllGather z2' -> z2g
  P7: conv2 aggregation, h2, e=elu(h2), v = e.fc_w + fc_b, mask,
      S = allreduce(sum(exp(v-48))), y = v - 48 - ln(S)

dma_gather needs int16 indices, so the gathered table is addressed through
two windows split at a core boundary (each < 32768 rows). Every per-core
chunk carries one guaranteed-zero row used for slot padding.
"""
import sys
import numpy as np

sys.path.insert(0, '/opt/trn_rl_repo')

import concourse.bass as bass
import concourse.bacc as bacc
import concourse.tile as tile
from concourse import mybir
from concourse import bass_utils
from concourse.masks import make_identity

F32 = mybir.dt.float32
I16 = mybir.dt.int16
AX = mybir.AxisListType
ALU = mybir.AluOpType
ACTF = mybir.ActivationFunctionType

P = 128
H = 64
BIG_NEG = -1e9
SOFTMAX_SHIFT = 48.0
GMAX_K = 48          # max slots per gather call (per-partition buffer budget)


def _wrap_idx(flat):
    """dma_gather index layout: [128, ceil(n/16)] int16, list wrapped into 16
    partitions (i -> [i%16, i//16]) and replicated across the 8 Q7 groups."""
    flat = np.asarray(flat, dtype=np.int64)
    n = flat.size
    s = (n + 15) // 16
    pad = np.full(s * 16, -1, dtype=np.int64)
    pad[:n] = flat
    assert pad.max() < 32768
    arr = pad.reshape(s, 16).T.astype(np.int16)     # [16, s]
    return np.tile(arr, (8, 1))                      # [128, s]


# ---------------------------------------------------------------------------
# Host preprocessing
# ---------------------------------------------------------------------------

def host_prep(x, edge_index, all_edge_index, s_mapping_index, e_mask, C):
    N, F = x.shape
    n_per = N // C
    NPC = ((n_per + P - 1) // P) * P
    NT = NPC // P
    NPC1 = NPC + 1                       # +1 zero row per core chunk
    ZLOC = NPC                           # zero row local index within chunk
    FP = ((F + P - 1) // P) * P
    C_LO = min(C - 1, 32767 // NPC1)
    assert C_LO >= 1 and (C - C_LO) * NPC1 <= 32768

    src = np.asarray(edge_index[0], dtype=np.int64)
    dst = np.asarray(edge_index[1], dtype=np.int64)
    deg = np.bincount(dst, minlength=N).astype(np.int64) + 1

    core_of = np.arange(N) // n_per
    is_lo_src = core_of[src] < C_LO
    nlo = np.bincount(dst[is_lo_src], minlength=N)
    nlo = nlo + (core_of < C_LO)                     # self loop
    nhi = deg - nlo

    def tile_cost(perm_all):
        tot = 0
        for arrs in (nlo, nhi):
            kt = np.zeros(NT, dtype=np.int64)
            for c in range(C):
                a = np.zeros(NPC, dtype=np.int64)
                a[:n_per] = arrs[perm_all[c]]
                kt = np.maximum(kt, a.reshape(NT, P).max(axis=1))
            tot += kt.sum()
        return tot

    cands = []
    for key in ('deg', 'lohi'):
        lp = np.empty((C, n_per), dtype=np.int64)
        for c in range(C):
            ids = np.arange(c * n_per, (c + 1) * n_per)
            if key == 'deg':
                o = np.argsort(-deg[ids], kind='stable')
            else:
                o = np.lexsort((-nhi[ids], -nlo[ids]))
            lp[c] = ids[o]
        cands.append((tile_cost(lp), lp))
    cands.sort(key=lambda t: t[0])
    lperm = cands[0][1]

    inv_row = np.empty(N, dtype=np.int64)
    for c in range(C):
        inv_row[lperm[c]] = c * NPC1 + np.arange(n_per)

    def per_tile_max(arrs):
        kt = np.zeros(NT, dtype=np.int64)
        for c in range(C):
            a = np.zeros(NPC, dtype=np.int64)
            a[:n_per] = arrs[lperm[c]]
            kt = np.maximum(kt, a.reshape(NT, P).max(axis=1))
        return kt

    K_lo = np.maximum(per_tile_max(nlo), 1)
    K_hi = np.maximum(per_tile_max(nhi), 1)

    es = np.argsort(dst, kind='stable')
    dst_sorted = dst[es]
    src_sorted = src[es]
    starts = np.searchsorted(dst_sorted, np.arange(N))
    ends = np.searchsorted(dst_sorted, np.arange(N) + 1)

    ZROW_LO = ZLOC                        # chunk 0's zero row (lo window)
    ZROW_HI = ZLOC                        # chunk C_LO's zero row (hi local)
    HI_BASE = C_LO * NPC1

    slots_lo = [[np.full((P, int(K_lo[t])), ZROW_LO, np.int64)
                 for t in range(NT)] for _ in range(C)]
    slots_hi = [[np.full((P, int(K_hi[t])), ZROW_HI, np.int64)
                 for t in range(NT)] for _ in range(C)]
    for c in range(C):
        for t in range(NT):
            base = t * P
            nreal = min(P, max(0, n_per - base))
            for pp in range(nreal):
                v = lperm[c, base + pp]
                srcs = src_sorted[starts[v]:ends[v]]
                allsrc = np.concatenate([[v], srcs])
                rows = inv_row[allsrc]
                lo = rows[rows < HI_BASE]
                hi = rows[rows >= HI_BASE] - HI_BASE
                slots_lo[c][t][pp, :lo.size] = lo
                slots_hi[c][t][pp, :hi.size] = hi

    groups = []          # (t0, t1, sum_klo, sum_khi)
    t0 = 0
    while t0 < NT:
        t1 = t0
        sk = 0
        while t1 < NT and sk + max(int(K_lo[t1]), int(K_hi[t1])) <= GMAX_K:
            sk += max(int(K_lo[t1]), int(K_hi[t1]))
            t1 += 1
        if t1 == t0:
            t1 = t0 + 1
        groups.append((t0, t1,
                       int(K_lo[t0:t1].sum()), int(K_hi[t0:t1].sum())))
        t0 = t1

    def build_wrapped(slotsets):
        per_core = []
        col_offs = None
        for c in range(C):
            parts = []
            for (a, b, _, _) in groups:
                flat = np.concatenate(
                    [slotsets[c][t].T.reshape(-1) for t in range(a, b)])
                # flat order within group: tile-major, slot k, partition p
                parts.append(_wrap_idx(flat))
            col_offs = np.cumsum([0] + [q.shape[1] for q in parts])
            per_core.append(np.concatenate(parts, axis=1))
        return np.stack(per_core), col_offs

    itbl_lo, lo_coffs = build_wrapped(slots_lo)
    itbl_hi, hi_coffs = build_wrapped(slots_hi)

    # x1 winner emulation (last-write-wins)
    map_vec = np.full(N, -1, dtype=np.int64)
    map_vec[np.asarray(s_mapping_index[1], dtype=np.int64)] = np.asarray(
        s_mapping_index[0], dtype=np.int64)
    gen = map_vec[np.asarray(all_edge_index[0], dtype=np.int64)]
    valid = gen >= 0
    tgt = np.asarray(all_edge_index[1], dtype=np.int64)[valid]
    genv = gen[valid]
    x1_src = np.full(N, -1, dtype=np.int64)
    if tgt.size:
        u_t, first_rev = np.unique(tgt[::-1], return_index=True)
        x1_src[u_t] = genv[tgt.size - 1 - first_rev]

    x1w_lo = np.empty((C, P, 8 * NT), dtype=np.int16)
    x1w_hi = np.empty((C, P, 8 * NT), dtype=np.int16)
    for c in range(C):
        flat_lo = np.full(NT * P, ZROW_LO, np.int64)
        flat_hi = np.full(NT * P, ZROW_HI, np.int64)
        for t in range(NT):
            base = t * P
            nreal = min(P, max(0, n_per - base))
            for pp in range(nreal):
                g = x1_src[lperm[c, base + pp]]
                if g >= 0:
                    r = inv_row[g]
                    if r < HI_BASE:
                        flat_lo[t * P + pp] = r
                    else:
                        flat_hi[t * P + pp] = r - HI_BASE
        x1w_lo[c] = _wrap_idx(flat_lo)
        x1w_hi[c] = _wrap_idx(flat_hi)

    degt = np.ones((C, P, NT), dtype=np.float32)
    keep = np.zeros((C, P, NT), dtype=np.float32)
    mneg = np.full((C, P, NT), np.float32(BIG_NEG), dtype=np.float32)
    maskf = np.asarray(e_mask).reshape(-1).astype(bool)
    for c in range(C):
        dp = np.ones(NPC, dtype=np.float32)
        dp[:n_per] = deg[lperm[c]].astype(np.float32)
        kp = np.zeros(NPC, dtype=np.float32)
        kp[:n_per] = (~maskf[lperm[c]]).astype(np.float32)
        mp = np.full(NPC, np.float32(BIG_NEG), dtype=np.float32)
        mp[:n_per] = np.where(maskf[lperm[c]], np.float32(BIG_NEG),
                              np.float32(0.0))
        degt[c] = dp.reshape(NT, P).T
        keep[c] = kp.reshape(NT, P).T
        mneg[c] = mp.reshape(NT, P).T

    xts = []
    xf = np.asarray(x, dtype=np.float32)
    for c in range(C):
        xt = np.zeros((FP, NPC), dtype=np.float32)
        xt[:F, :n_per] = xf[lperm[c]].T
        xts.append(xt)

    tot_slots = int((K_lo.sum() + K_hi.sum()) * P)
    real_slots = int(E + N) // C if (E := len(src)) else 0
    meta = dict(N=N, F=F, C=C, n_per=n_per, NPC=NPC, NPC1=NPC1, NT=NT,
                tot_slots=tot_slots, real_slots=real_slots,
                FP=FP, C_LO=C_LO, HI_BASE=HI_BASE,
                K_lo=K_lo, K_hi=K_hi, groups=groups,
                lo_coffs=list(lo_coffs), hi_coffs=list(hi_coffs),
                SUMW_LO=itbl_lo.shape[2], SUMW_HI=itbl_hi.shape[2],
                lperm=lperm)
    return dict(xts=xts, itbl_lo=itbl_lo, itbl_hi=itbl_hi,
                x1w_lo=x1w_lo, x1w_hi=x1w_hi,
                degt=degt, keep=keep, mneg=mneg), meta


def host_prep_weights(conv1_w, conv1_b, conv2_w, conv2_b, fc_w, fc_b, meta):
    FP = meta['FP']
    F = meta['F']
    w1 = np.zeros((FP, H), dtype=np.float32)
    w1[:F] = np.asarray(conv1_w, dtype=np.float32)
    w2 = np.asarray(conv2_w, dtype=np.float32)
    b1r = np.broadcast_to(np.asarray(conv1_b, np.float32), (P, H)).copy()
    b2r = np.broadcast_to(np.asarray(conv2_b, np.float32), (P, H)).copy()
    fcwr = np.broadcast_to(np.asarray(fc_w, np.float32).reshape(1, H),
                           (P, H)).copy()
    fcbr = np.full((P, 1), np.float32(np.asarray(fc_b).reshape(-1)[0]),
                   np.float32)
    return w1, w2, b1r, b2r, fcwr, fcbr


# ---------------------------------------------------------------------------
# Device kernel
# ---------------------------------------------------------------------------

def build_kernel(meta, mm_dtype=F32, debug_taps=False, skip_b1=False,
                 skip_b2=False):
    C = meta['C']
    NPC = meta['NPC']
    NPC1 = meta['NPC1']
    NT = meta['NT']
    FP = meta['FP']
    C_LO = meta['C_LO']
    K_lo = meta['K_lo']
    K_hi = meta['K_hi']
    groups = meta['groups']
    lo_coffs = meta['lo_coffs']
    hi_coffs = meta['hi_coffs']
    SUMW_LO = meta['SUMW_LO']
    SUMW_HI = meta['SUMW_HI']
    NF = FP // P
    MBLK = 512
    n_mblk = (NPC + MBLK - 1) // MBLK
    NLO = C_LO * NPC1
    NHI = (C - C_LO) * NPC1
    GK_LO = max(g[2] for g in groups)
    GK_HI = max(g[3] for g in groups)

    nc = bacc.Bacc("TRN2", target_bir_lowering=False, debug=False,
                   num_devices=C, num_swdge_queues=4)

    xt_d = nc.dram_tensor("xt", [FP, NPC], mm_dtype, kind="ExternalInput")
    w1_d = nc.dram_tensor("w1", [FP, H], mm_dtype, kind="ExternalInput")
    w2_d = nc.dram_tensor("w2", [2 * H, H], F32, kind="ExternalInput")
    b1_d = nc.dram_tensor("b1r", [P, H], F32, kind="ExternalInput")
    b2_d = nc.dram_tensor("b2r", [P, H], F32, kind="ExternalInput")
    fcw_d = nc.dram_tensor("fcwr", [P, H], F32, kind="ExternalInput")
    fcb_d = nc.dram_tensor("fcbr", [P, 1], F32, kind="ExternalInput")
    ilo_d = nc.dram_tensor("itbl_lo", [P, SUMW_LO], I16, kind="ExternalInput")
    ihi_d = nc.dram_tensor("itbl_hi", [P, SUMW_HI], I16, kind="ExternalInput")
    x1lo_d = nc.dram_tensor("x1w_lo", [P, 8 * NT], I16, kind="ExternalInput")
    x1hi_d = nc.dram_tensor("x1w_hi", [P, 8 * NT], I16, kind="ExternalInput")
    degt_d = nc.dram_tensor("degt", [P, NT], F32, kind="ExternalInput")
    keep_d = nc.dram_tensor("keep", [P, NT], F32, kind="ExternalInput")
    mneg_d = nc.dram_tensor("mneg", [P, NT], F32, kind="ExternalInput")
    y_d = nc.dram_tensor("y", [P, NT], F32, kind="ExternalOutput")
    if debug_taps:
        z1dbg_d = nc.dram_tensor("z1dbg", [NPC1, H], F32, kind="ExternalOutput")
        udbg_d = nc.dram_tensor("udbg", [NPC1, H], F32, kind="ExternalOutput")
        z2dbg_d = nc.dram_tensor("z2dbg", [NPC1, H], F32, kind="ExternalOutput")
        vdbg_d = nc.dram_tensor("vdbg", [P, NT], F32, kind="ExternalOutput")
        gdbg_d = nc.dram_tensor("gdbg", [P, groups[0][2] * H], F32,
                                kind="ExternalOutput")

    rg = [list(range(C))]

    with tile.TileContext(nc) as tc:
        with tc.tile_pool(name="const", bufs=1) as cpool, \
             tc.tile_pool(name="xin", bufs=4) as xpool, \
             tc.tile_pool(name="work", bufs=2) as wpool, \
             tc.tile_pool(name="gath", bufs=3) as gpool, \
             tc.tile_pool(name="big", bufs=1) as bpool, \
             tc.tile_pool(name="ps", bufs=4, space="PSUM") as pspool, \
             tc.tile_pool(name="psz", bufs=2, space="PSUM") as pszpool, \
             tc.tile_pool(name="dram", bufs=1, space="DRAM") as dpool:

            z1b = dpool.tile([NPC1, H], F32)
            ub = dpool.tile([NPC1, H], F32)
            z2b = dpool.tile([NPC1, H], F32)
            ash = "Shared" if C > 4 else "Local"
            z1g = nc.dram_tensor("z1g_sh", [C * NPC1, H], F32,
                                 kind="Internal", addr_space=ash).ap()
            ug = nc.dram_tensor("ug_sh", [C * NPC1, H], F32,
                                kind="Internal", addr_space=ash).ap()
            z2g = nc.dram_tensor("z2g_sh", [C * NPC1, H], F32,
                                 kind="Internal", addr_space=ash).ap()
            sj_in = dpool.tile([1, 1], F32)
            sj_out = nc.dram_tensor("sj_sh", [1, 1], F32,
                                    kind="Internal", addr_space=ash).ap()

            ident = cpool.tile([P, P], F32)
            make_identity(nc, ident[:])
            w1_sb = cpool.tile([P, NF * H], mm_dtype)
            nc.sync.dma_start(
                w1_sb[:], w1_d.ap().rearrange("(a p) h -> p a h", p=P))
            w2_sb = cpool.tile([P, H], F32)
            nc.sync.dma_start(w2_sb[:], w2_d.ap())
            b1_sb = cpool.tile([P, H], F32)
            nc.sync.dma_start(b1_sb[:], b1_d.ap())
            b2_sb = cpool.tile([P, H], F32)
            nc.sync.dma_start(b2_sb[:], b2_d.ap())
            fcw_sb = cpool.tile([P, H], F32)
            nc.sync.dma_start(fcw_sb[:], fcw_d.ap())
            fcb_sb = cpool.tile([P, 1], F32)
            nc.sync.dma_start(fcb_sb[:], fcb_d.ap())
            ilo_sb = cpool.tile([P, SUMW_LO], I16)
            nc.sync.dma_start(ilo_sb[:], ilo_d.ap())
            ihi_sb = cpool.tile([P, SUMW_HI], I16)
            nc.sync.dma_start(ihi_sb[:], ihi_d.ap())
            x1lo_sb = cpool.tile([P, 8 * NT], I16)
            nc.sync.dma_start(x1lo_sb[:], x1lo_d.ap())
            x1hi_sb = cpool.tile([P, 8 * NT], I16)
            nc.sync.dma_start(x1hi_sb[:], x1hi_d.ap())
            degt_sb = cpool.tile([P, NT], F32)
            nc.sync.dma_start(degt_sb[:], degt_d.ap())
            keep_sb = cpool.tile([P, NT], F32)
            nc.sync.dma_start(keep_sb[:], keep_d.ap())
            mneg_sb = cpool.tile([P, NT], F32)
            nc.sync.dma_start(mneg_sb[:], mneg_d.ap())

            dinv_sb = cpool.tile([P, NT], F32)
            nc.vector.reciprocal(dinv_sb[:], degt_sb[:])
            nc.scalar.activation(dinv_sb[:], dinv_sb[:], ACTF.Sqrt)

            zrow_sb = cpool.tile([1, H], F32)
            nc.vector.memset(zrow_sb[:], 0.0)
            nc.sync.dma_start(z1b[NPC:NPC + 1, :], zrow_sb[:])
            nc.sync.dma_start(ub[NPC:NPC + 1, :], zrow_sb[:])
            nc.sync.dma_start(z2b[NPC:NPC + 1, :], zrow_sb[:])

            ones_sb = cpool.tile([P, 1], F32)
            nc.vector.memset(ones_sb[:], 1.0)
            ones_row = cpool.tile([1, P], F32)
            nc.vector.memset(ones_row[:], 1.0)
            neg48_sb = cpool.tile([P, 1], F32)
            nc.vector.memset(neg48_sb[:], -SOFTMAX_SHIFT)

            u_loc = bpool.tile([P, NT * 2 * H], F32)   # [u | x1u] interleaved

            def elu_into(dst_ap, src_ap, tmp_pool, fd):
                mn = tmp_pool.tile([P, fd], F32, tag="elu_mn")
                ex = tmp_pool.tile([P, fd], F32, tag="elu_ex")
                nc.vector.tensor_scalar(out=mn[:], in0=src_ap, scalar1=0.0,
                                        scalar2=None, op0=ALU.min)
                nc.scalar.activation(ex[:], mn[:], ACTF.Exp)
                nc.vector.tensor_scalar(out=mn[:], in0=src_ap, scalar1=0.0,
                                        scalar2=None, op0=ALU.max)
                nc.vector.tensor_scalar(out=ex[:], in0=ex[:], scalar1=-1.0,
                                        scalar2=None, op0=ALU.add)
                nc.vector.tensor_tensor(out=dst_ap, in0=mn[:], in1=ex[:],
                                        op=ALU.add)

            def agg_layer(src_g, out_cb, tap=False):
                qn = [0]
                for gi, (a, b, sklo, skhi) in enumerate(groups):
                    glo = gpool.tile([P, GK_LO * H], F32, tag="glo")
                    ghi = gpool.tile([P, GK_HI * H], F32, tag="ghi")
                    nlo_idx = sklo * P
                    nhi_idx = skhi * P
                    nc.gpsimd.dma_gather(
                        out_ap=glo[:, :sklo * H].rearrange(
                            "p (c h) -> p c h", h=H),
                        in_ap=src_g[0:NLO, :],
                        idxs_ap=ilo_sb[:, lo_coffs[gi]:lo_coffs[gi + 1]],
                        num_idxs=nlo_idx, num_idxs_reg=nlo_idx, elem_size=H,
                        single_packet=False, queue_num=(2 * gi) % 4)
                    nc.gpsimd.dma_gather(
                        out_ap=ghi[:, :skhi * H].rearrange(
                            "p (c h) -> p c h", h=H),
                        in_ap=src_g[NLO:NLO + NHI, :],
                        idxs_ap=ihi_sb[:, hi_coffs[gi]:hi_coffs[gi + 1]],
                        num_idxs=nhi_idx, num_idxs_reg=nhi_idx, elem_size=H,
                        single_packet=False, queue_num=(2 * gi + 1) % 4)
                    if tap and gi == 0:
                        nc.sync.dma_start(gdbg_d.ap()[:], glo[:, :sklo * H])
                    ko = 0
                    kho = 0
                    for t in range(a, b):
                        klo_t = int(K_lo[t])
                        khi_t = int(K_hi[t])
                        ra = wpool.tile([P, H], F32, tag="ra")
                        nc.vector.tensor_reduce(
                            out=ra[:],
                            in_=glo[:, ko * H:(ko + klo_t) * H].rearrange(
                                "p (k h) -> p h k", k=klo_t),
                            op=ALU.add, axis=AX.X)
                        rb = wpool.tile([P, H], F32, tag="rb")
                        nc.vector.tensor_reduce(
                            out=rb[:],
                            in_=ghi[:, kho * H:(kho + khi_t) * H].rearrange(
                                "p (k h) -> p h k", k=khi_t),
                            op=ALU.add, axis=AX.X)
                        nc.vector.tensor_tensor(out=ra[:], in0=ra[:],
                                                in1=rb[:], op=ALU.add)
                        agg = wpool.tile([P, H], F32, tag="agg")
                        nc.vector.tensor_scalar(
                            out=agg[:], in0=ra[:],
                            scalar1=dinv_sb[:, t:t + 1], scalar2=None,
                            op0=ALU.mult)
                        out_cb(t, agg)
                        ko += klo_t
                        kho += khi_t

            # ================= P1: z1 = x @ W1 ===========================
            for mp in range(0, n_mblk, 2):
                nmb = min(2, n_mblk - mp)
                w0 = min(MBLK * nmb, NPC - mp * MBLK)
                zt_list = [pszpool.tile([H, MBLK], F32, tag=f"zt{j}",
                                        name=f"ztp{j}")
                           for j in range(nmb)]
                for f in range(NF):
                    xt_sb = xpool.tile([P, 2 * MBLK], mm_dtype, tag="xt")
                    nc.sync.dma_start(
                        xt_sb[:, :w0],
                        xt_d.ap()[f * P:(f + 1) * P,
                                  mp * MBLK:mp * MBLK + w0])
                    for j in range(nmb):
                        mw = min(MBLK, NPC - (mp + j) * MBLK)
                        nc.tensor.matmul(
                            zt_list[j][:, :mw],
                            lhsT=w1_sb[:, f * H:(f + 1) * H],
                            rhs=xt_sb[:, j * MBLK:j * MBLK + mw],
                            start=(f == 0), stop=(f == NF - 1))
                for j in range(nmb):
                    m = mp + j
                    mw = min(MBLK, NPC - m * MBLK)
                    zt_sb = wpool.tile([H, MBLK], F32, tag="zt_sb")
                    nc.vector.tensor_copy(zt_sb[:, :mw], zt_list[j][:, :mw])
                    for k in range(mw // P):
                        t = m * (MBLK // P) + k
                        tr_ps = pspool.tile([P, H], F32, tag="pss")
                        nc.tensor.transpose(
                            tr_ps[:], zt_sb[:, k * P:(k + 1) * P],
                            ident[:H, :H])
                        z1p_sb = wpool.tile([P, H], F32, tag="z1p")
                        nc.vector.tensor_scalar(
                            out=z1p_sb[:], in0=tr_ps[:],
                            scalar1=dinv_sb[:, t:t + 1], scalar2=None,
                            op0=ALU.mult)
                        nc.sync.dma_start(z1b[t * P:(t + 1) * P, :],
                                          z1p_sb[:])

            # ================= P2: AllGather z1' =========================
            nc.gpsimd.collective_compute(
                "AllGather", ALU.bypass, replica_groups=rg,
                ins=[z1b.opt()], outs=[z1g[:, :]])

            # ================= P3: conv1 agg + elu =======================
            def p3_out(t, agg):
                if not skip_b1:
                    nc.vector.tensor_tensor(out=agg[:], in0=agg[:],
                                            in1=b1_sb[:], op=ALU.add)
                udst = u_loc[:, t * 2 * H: t * 2 * H + H]
                elu_into(udst, agg[:], wpool, H)
                nc.sync.dma_start(ub[t * P:(t + 1) * P, :], udst)
            agg_layer(z1g, p3_out, tap=debug_taps)

            # ================= P4: AllGather u ===========================
            nc.gpsimd.collective_compute(
                "AllGather", ALU.bypass, replica_groups=rg,
                ins=[ub.opt()], outs=[ug[:, :]])

            # ================= P5: x1 gather + z2 ========================
            xga = bpool.tile([P, NT * H], F32)
            xgb = bpool.tile([P, NT * H], F32)
            nidx = NT * P
            nc.gpsimd.dma_gather(
                out_ap=xga[:].rearrange("p (c h) -> p c h", h=H),
                in_ap=ug[0:NLO, :], idxs_ap=x1lo_sb[:, :],
                num_idxs=nidx, num_idxs_reg=nidx, elem_size=H,
                single_packet=False, queue_num=0)
            nc.gpsimd.dma_gather(
                out_ap=xgb[:].rearrange("p (c h) -> p c h", h=H),
                in_ap=ug[NLO:NLO + NHI, :], idxs_ap=x1hi_sb[:, :],
                num_idxs=nidx, num_idxs_reg=nidx, elem_size=H,
                single_packet=False, queue_num=1)
            nc.vector.tensor_tensor(
                out=u_loc[:].rearrange("p (t h) -> p t h", t=2 * NT)[:, 1::2, :],
                in0=xga[:].rearrange("p (t h) -> p t h", t=NT),
                in1=xgb[:].rearrange("p (t h) -> p t h", t=NT),
                op=ALU.add)
            for t in range(NT):
                hT_ps = pspool.tile([P, P], F32, tag="pss")
                nc.tensor.transpose(
                    hT_ps[:], u_loc[:, t * 2 * H:(t + 1) * 2 * H], ident[:])
                hT_sb = wpool.tile([P, P], F32, tag="hT_sb")
                nc.vector.tensor_copy(hT_sb[:], hT_ps[:])
                z2_ps = pspool.tile([P, H], F32, tag="pss")
                nc.tensor.matmul(z2_ps[:], lhsT=hT_sb[:], rhs=w2_sb[:],
                                 start=True, stop=True)
                z2p_sb = wpool.tile([P, H], F32, tag="z2p")
                nc.vector.tensor_scalar(
                    out=z2p_sb[:], in0=z2_ps[:], scalar1=dinv_sb[:, t:t + 1],
                    scalar2=None, op0=ALU.mult)
                nc.sync.dma_start(z2b[t * P:(t + 1) * P, :], z2p_sb[:])

            # ================= P6: AllGather z2' =========================
            nc.gpsimd.collective_compute(
                "AllGather", ALU.bypass, replica_groups=rg,
                ins=[z2b.opt()], outs=[z2g[:, :]])

            # ================= P7: conv2 agg + head ======================
            vbuf = bpool.tile([P, NT], F32)

            def p7_out(t, agg):
                if not skip_b2:
                    nc.vector.tensor_tensor(out=agg[:], in0=agg[:],
                                            in1=b2_sb[:], op=ALU.add)
                e2 = wpool.tile([P, H], F32, tag="e2")
                elu_into(e2[:], agg[:], wpool, H)
                nc.vector.tensor_tensor(out=e2[:], in0=e2[:], in1=fcw_sb[:],
                                        op=ALU.mult)
                nc.vector.tensor_reduce(out=vbuf[:, t:t + 1], in_=e2[:],
                                        op=ALU.add, axis=AX.X)
            agg_layer(z2g, p7_out)

            nc.vector.tensor_tensor(out=vbuf[:], in0=vbuf[:], in1=keep_sb[:],
                                    op=ALU.mult)
            nc.vector.tensor_tensor(out=vbuf[:], in0=vbuf[:], in1=mneg_sb[:],
                                    op=ALU.add)
            es = bpool.tile([P, NT], F32)
            acc = wpool.tile([P, 1], F32, tag="acc")
            nc.scalar.activation(es[:], vbuf[:], ACTF.Exp,
                                 bias=neg48_sb[:], scale=1.0,
                                 accum_out=acc[:])
            s_ps = pspool.tile([1, 1], F32, tag="pss")
            nc.tensor.matmul(s_ps[:], lhsT=acc[:], rhs=ones_sb[:],
                             start=True, stop=True)
            s_sb = wpool.tile([1, 1], F32, tag="s_sb")
            nc.vector.tensor_copy(s_sb[:], s_ps[:])
            nc.sync.dma_start(sj_in[:], s_sb[:])
            nc.gpsimd.collective_compute(
                "AllReduce", ALU.add, replica_groups=rg,
                ins=[sj_in.opt()], outs=[sj_out[:, :]])
            s2_sb = wpool.tile([1, 1], F32, tag="s2_sb")
            nc.sync.dma_start(s2_sb[:], sj_out[:, :])
            lnS = wpool.tile([1, 1], F32, tag="lnS")
            nc.scalar.activation(lnS[:], s2_sb[:], ACTF.Ln)
            b_ps = pspool.tile([P, 1], F32, tag="pss")
            nc.tensor.matmul(b_ps[:], lhsT=ones_row[:], rhs=lnS[:],
                             start=True, stop=True)
            bias_sb = wpool.tile([P, 1], F32, tag="bias_sb")
            nc.vector.tensor_scalar(out=bias_sb[:], in0=b_ps[:],
                                    scalar1=-1.0, scalar2=-SOFTMAX_SHIFT,
                                    op0=ALU.mult, op1=ALU.add)
            y_sb = bpool.tile([P, NT], F32)
            nc.vector.tensor_tensor(out=y_sb[:], in0=vbuf[:],
                                    in1=bias_sb[:].to_broadcast([P, NT]),
                                    op=ALU.add)
            nc.sync.dma_start(y_d.ap()[:], y_sb[:])
            if debug_taps:
                nc.sync.dma_start(z1dbg_d.ap()[:], z1b[:])
                nc.sync.dma_start(udbg_d.ap()[:], ub[:])
                nc.sync.dma_start(z2dbg_d.ap()[:], z2b[:])
                nc.sync.dma_start(vdbg_d.ap()[:], vbuf[:])

    nc.compile()
    return nc


# ---------------------------------------------------------------------------
# Full flow
# ---------------------------------------------------------------------------

def run(x, edge_index, all_edge_index, s_mapping_index, e_mask,
        conv1_w, conv1_b, conv2_w, conv2_b, fc_w, fc_b,
        C=8, mm_dtype=F32, trace=False, nc_cache=None, debug_taps=False,
        **rbk_kwargs):
    tabs, meta = host_prep(
        x, edge_index, all_edge_index, s_mapping_index, e_mask, C)
    w1, w2, b1r, b2r, fcwr, fcbr = host_prep_weights(
        conv1_w, conv1_b, conv2_w, conv2_b, fc_w, fc_b, meta)
    fcb_val = np.float32(np.asarray(fc_b).reshape(-1)[0])
    for c in range(C):
        tabs['mneg'][c] = (tabs['mneg'][c]
                           + fcb_val * tabs['keep'][c]).astype(np.float32)
    skip_b1 = bool(np.all(np.asarray(conv1_b) == 0))
    skip_b2 = bool(np.all(np.asarray(conv2_b) == 0))

    if nc_cache is not None and 'nc' in nc_cache:
        nc = nc_cache['nc']
    else:
        nc = build_kernel(meta, mm_dtype=mm_dtype, debug_taps=debug_taps,
                          skip_b1=skip_b1, skip_b2=skip_b2)
        if nc_cache is not None:
            nc_cache['nc'] = nc

    in_maps = []
    for c in range(C):
        in_maps.append(dict(
            xt=tabs['xts'][c], w1=w1, w2=w2, b1r=b1r, b2r=b2r, fcwr=fcwr,
            fcbr=fcbr, itbl_lo=tabs['itbl_lo'][c], itbl_hi=tabs['itbl_hi'][c],
            x1w_lo=tabs['x1w_lo'][c], x1w_hi=tabs['x1w_hi'][c],
            degt=tabs['degt'][c], keep=tabs['keep'][c], mneg=tabs['mneg'][c]))
    res = bass_utils.run_bass_kernel_spmd(
        nc, in_maps, core_ids=list(range(C)), trace=trace, **rbk_kwargs)

    N = meta['N']
    n_per = meta['n_per']
    out = np.empty((N, 1), dtype=np.float32)
    for c in range(C):
        yc = res.results[c]['y']
        flat = yc.T.reshape(-1)
        out[meta['lperm'][c], 0] = flat[:n_per]
    return out, res, meta


# ---------------------------------------------------------------------------
# Harness entry point
# ---------------------------------------------------------------------------

_NC_CACHE = {}


def kernel(**inputs):
    """Full (unsharded) inputs -> full [N, 1] float32 output."""
    out, _res, _meta = run(
        x=np.asarray(inputs['x'], dtype=np.float32),
        edge_index=np.asarray(inputs['edge_index']),
        all_edge_index=np.asarray(inputs['all_edge_index']),
        s_mapping_index=np.asarray(inputs['s_mapping_index']),
        e_mask=np.asarray(inputs['e_mask']),
        conv1_w=np.asarray(inputs['conv1_w'], dtype=np.float32),
        conv1_b=np.asarray(inputs['conv1_b'], dtype=np.float32),
        conv2_w=np.asarray(inputs['conv2_w'], dtype=np.float32),
        conv2_b=np.asarray(inputs['conv2_b'], dtype=np.float32),
        fc_w=np.asarray(inputs['fc_w'], dtype=np.float32),
        fc_b=np.asarray(inputs['fc_b'], dtype=np.float32),
        C=8, mm_dtype=mybir.dt.float32r, trace=False, nc_cache=_NC_CACHE)
    return out



# revision 11
# speedup vs baseline: 1.4428x; 1.4428x over previous
"""GCN EndNodeSelector Bass kernel for TRN2, 8-core SPMD.

Pipeline (per core, nodes row-sharded, degree-sorted within core):
  P1: z1 = x @ W1 (PE, xT streamed from DRAM), z1' = dinv * z1
  P2: AllGather z1' -> z1g (per-core chunks of NPC+1 rows; last row zero)
  P3: conv1 aggregation via dma_gather (LO/HI windows) + strided
      tensor_reduce; h1 = dinv * sum + b1 ; u = elu(h1)
  P4: AllGather u -> ug
  P5: x1 gather from ug (mapping winners), hcat=[u|x1u], z2 = hcat @ W2
  P6: AllGather z2' -> z2g
  P7: conv2 aggregation, h2, e=elu(h2), v = e.fc_w + fc_b, mask,
      S = allreduce(sum(exp(v-48))), y = v - 48 - ln(S)

dma_gather needs int16 indices, so the gathered table is addressed through
two windows split at a core boundary (each < 32768 rows). Every per-core
chunk carries one guaranteed-zero row used for slot padding.
"""
import sys
import numpy as np

sys.path.insert(0, '/opt/trn_rl_repo')

import concourse.bass as bass
import concourse.bacc as bacc
import concourse.tile as tile
from concourse import mybir
from concourse import bass_utils
from concourse.masks import make_identity

F32 = mybir.dt.float32
I16 = mybir.dt.int16
AX = mybir.AxisListType
ALU = mybir.AluOpType
ACTF = mybir.ActivationFunctionType

P = 128
H = 64
BIG_NEG = -1e9
SOFTMAX_SHIFT = 48.0
GMAX_K = 48          # max slots per gather call (per-partition buffer budget)


def _wrap_idx(flat):
    """dma_gather index layout: [128, ceil(n/16)] int16, list wrapped into 16
    partitions (i -> [i%16, i//16]) and replicated across the 8 Q7 groups."""
    flat = np.asarray(flat, dtype=np.int64)
    n = flat.size
    s = (n + 15) // 16
    pad = np.full(s * 16, -1, dtype=np.int64)
    pad[:n] = flat
    assert pad.max() < 32768
    arr = pad.reshape(s, 16).T.astype(np.int16)     # [16, s]
    return np.tile(arr, (8, 1))                      # [128, s]


# ---------------------------------------------------------------------------
# Host preprocessing
# ---------------------------------------------------------------------------

def host_prep(x, edge_index, all_edge_index, s_mapping_index, e_mask, C):
    N, F = x.shape
    n_per = N // C
    NPC = ((n_per + P - 1) // P) * P
    NT = NPC // P
    NPC1 = NPC + 1                       # +1 zero row per core chunk
    ZLOC = NPC                           # zero row local index within chunk
    FP = ((F + P - 1) // P) * P
    C_LO = min(C - 1, 32767 // NPC1)
    assert C_LO >= 1 and (C - C_LO) * NPC1 <= 32768

    src = np.asarray(edge_index[0], dtype=np.int64)
    dst = np.asarray(edge_index[1], dtype=np.int64)
    deg = np.bincount(dst, minlength=N).astype(np.int64) + 1

    core_of = np.arange(N) // n_per
    is_lo_src = core_of[src] < C_LO
    nlo = np.bincount(dst[is_lo_src], minlength=N)
    nlo = nlo + (core_of < C_LO)                     # self loop
    nhi = deg - nlo

    def tile_cost(perm_all):
        tot = 0
        for arrs in (nlo, nhi):
            kt = np.zeros(NT, dtype=np.int64)
            for c in range(C):
                a = np.zeros(NPC, dtype=np.int64)
                a[:n_per] = arrs[perm_all[c]]
                kt = np.maximum(kt, a.reshape(NT, P).max(axis=1))
            tot += kt.sum()
        return tot

    cands = []
    for key in ('deg', 'lohi'):
        lp = np.empty((C, n_per), dtype=np.int64)
        for c in range(C):
            ids = np.arange(c * n_per, (c + 1) * n_per)
            if key == 'deg':
                o = np.argsort(-deg[ids], kind='stable')
            else:
                o = np.lexsort((-nhi[ids], -nlo[ids]))
            lp[c] = ids[o]
        cands.append((tile_cost(lp), lp))
    cands.sort(key=lambda t: t[0])
    lperm = cands[0][1]

    inv_row = np.empty(N, dtype=np.int64)
    for c in range(C):
        inv_row[lperm[c]] = c * NPC1 + np.arange(n_per)

    def per_tile_max(arrs):
        kt = np.zeros(NT, dtype=np.int64)
        for c in range(C):
            a = np.zeros(NPC, dtype=np.int64)
            a[:n_per] = arrs[lperm[c]]
            kt = np.maximum(kt, a.reshape(NT, P).max(axis=1))
        return kt

    K_lo = np.maximum(per_tile_max(nlo), 1)
    K_hi = np.maximum(per_tile_max(nhi), 1)

    es = np.argsort(dst, kind='stable')
    dst_sorted = dst[es]
    src_sorted = src[es]
    starts = np.searchsorted(dst_sorted, np.arange(N))
    ends = np.searchsorted(dst_sorted, np.arange(N) + 1)

    ZROW_LO = ZLOC                        # chunk 0's zero row (lo window)
    ZROW_HI = ZLOC                        # chunk C_LO's zero row (hi local)
    HI_BASE = C_LO * NPC1

    slots_lo = [[np.full((P, int(K_lo[t])), ZROW_LO, np.int64)
                 for t in range(NT)] for _ in range(C)]
    slots_hi = [[np.full((P, int(K_hi[t])), ZROW_HI, np.int64)
                 for t in range(NT)] for _ in range(C)]
    for c in range(C):
        for t in range(NT):
            base = t * P
            nreal = min(P, max(0, n_per - base))
            for pp in range(nreal):
                v = lperm[c, base + pp]
                srcs = src_sorted[starts[v]:ends[v]]
                allsrc = np.concatenate([[v], srcs])
                rows = inv_row[allsrc]
                lo = rows[rows < HI_BASE]
                hi = rows[rows >= HI_BASE] - HI_BASE
                slots_lo[c][t][pp, :lo.size] = lo
                slots_hi[c][t][pp, :hi.size] = hi

    groups = []          # (t0, t1, sum_klo, sum_khi)
    t0 = 0
    while t0 < NT:
        t1 = t0
        sk = 0
        while t1 < NT and sk + max(int(K_lo[t1]), int(K_hi[t1])) <= GMAX_K:
            sk += max(int(K_lo[t1]), int(K_hi[t1]))
            t1 += 1
        if t1 == t0:
            t1 = t0 + 1
        groups.append((t0, t1,
                       int(K_lo[t0:t1].sum()), int(K_hi[t0:t1].sum())))
        t0 = t1

    def build_wrapped(slotsets):
        per_core = []
        col_offs = None
        for c in range(C):
            parts = []
            for (a, b, _, _) in groups:
                flat = np.concatenate(
                    [slotsets[c][t].T.reshape(-1) for t in range(a, b)])
                # flat order within group: tile-major, slot k, partition p
                parts.append(_wrap_idx(flat))
            col_offs = np.cumsum([0] + [q.shape[1] for q in parts])
            per_core.append(np.concatenate(parts, axis=1))
        return np.stack(per_core), col_offs

    itbl_lo, lo_coffs = build_wrapped(slots_lo)
    itbl_hi, hi_coffs = build_wrapped(slots_hi)

    # x1 winner emulation (last-write-wins)
    map_vec = np.full(N, -1, dtype=np.int64)
    map_vec[np.asarray(s_mapping_index[1], dtype=np.int64)] = np.asarray(
        s_mapping_index[0], dtype=np.int64)
    gen = map_vec[np.asarray(all_edge_index[0], dtype=np.int64)]
    valid = gen >= 0
    tgt = np.asarray(all_edge_index[1], dtype=np.int64)[valid]
    genv = gen[valid]
    x1_src = np.full(N, -1, dtype=np.int64)
    if tgt.size:
        u_t, first_rev = np.unique(tgt[::-1], return_index=True)
        x1_src[u_t] = genv[tgt.size - 1 - first_rev]

    x1w_lo = np.empty((C, P, 8 * NT), dtype=np.int16)
    x1w_hi = np.empty((C, P, 8 * NT), dtype=np.int16)
    for c in range(C):
        flat_lo = np.full(NT * P, ZROW_LO, np.int64)
        flat_hi = np.full(NT * P, ZROW_HI, np.int64)
        for t in range(NT):
            base = t * P
            nreal = min(P, max(0, n_per - base))
            for pp in range(nreal):
                g = x1_src[lperm[c, base + pp]]
                if g >= 0:
                    r = inv_row[g]
                    if r < HI_BASE:
                        flat_lo[t * P + pp] = r
                    else:
                        flat_hi[t * P + pp] = r - HI_BASE
        x1w_lo[c] = _wrap_idx(flat_lo)
        x1w_hi[c] = _wrap_idx(flat_hi)

    degt = np.ones((C, P, NT), dtype=np.float32)
    keep = np.zeros((C, P, NT), dtype=np.float32)
    mneg = np.full((C, P, NT), np.float32(BIG_NEG), dtype=np.float32)
    maskf = np.asarray(e_mask).reshape(-1).astype(bool)
    for c in range(C):
        dp = np.ones(NPC, dtype=np.float32)
        dp[:n_per] = deg[lperm[c]].astype(np.float32)
        kp = np.zeros(NPC, dtype=np.float32)
        kp[:n_per] = (~maskf[lperm[c]]).astype(np.float32)
        mp = np.full(NPC, np.float32(BIG_NEG), dtype=np.float32)
        mp[:n_per] = np.where(maskf[lperm[c]], np.float32(BIG_NEG),
                              np.float32(0.0))
        degt[c] = dp.reshape(NT, P).T
        keep[c] = kp.reshape(NT, P).T
        mneg[c] = mp.reshape(NT, P).T

    xts = []
    xf = np.asarray(x, dtype=np.float32)
    for c in range(C):
        xt = np.zeros((FP, NPC), dtype=np.float32)
        xt[:F, :n_per] = xf[lperm[c]].T
        xts.append(xt)

    tot_slots = int((K_lo.sum() + K_hi.sum()) * P)
    real_slots = int(E + N) // C if (E := len(src)) else 0
    meta = dict(N=N, F=F, C=C, n_per=n_per, NPC=NPC, NPC1=NPC1, NT=NT,
                tot_slots=tot_slots, real_slots=real_slots,
                FP=FP, C_LO=C_LO, HI_BASE=HI_BASE,
                K_lo=K_lo, K_hi=K_hi, groups=groups,
                lo_coffs=list(lo_coffs), hi_coffs=list(hi_coffs),
                SUMW_LO=itbl_lo.shape[2], SUMW_HI=itbl_hi.shape[2],
                lperm=lperm)
    return dict(xts=xts, itbl_lo=itbl_lo, itbl_hi=itbl_hi,
                x1w_lo=x1w_lo, x1w_hi=x1w_hi,
                degt=degt, keep=keep, mneg=mneg), meta


def host_prep_weights(conv1_w, conv1_b, conv2_w, conv2_b, fc_w, fc_b, meta):
    FP = meta['FP']
    F = meta['F']
    w1 = np.zeros((FP, H), dtype=np.float32)
    w1[:F] = np.asarray(conv1_w, dtype=np.float32)
    w2 = np.asarray(conv2_w, dtype=np.float32)
    b1r = np.broadcast_to(np.asarray(conv1_b, np.float32), (P, H)).copy()
    b2r = np.broadcast_to(np.asarray(conv2_b, np.float32), (P, H)).copy()
    fcwr = np.broadcast_to(np.asarray(fc_w, np.float32).reshape(1, H),
                           (P, H)).copy()
    fcbr = np.full((P, 1), np.float32(np.asarray(fc_b).reshape(-1)[0]),
                   np.float32)
    return w1, w2, b1r, b2r, fcwr, fcbr


# ---------------------------------------------------------------------------
# Device kernel
# ---------------------------------------------------------------------------

def build_kernel(meta, mm_dtype=F32, debug_taps=False, skip_b1=False,
                 skip_b2=False):
    C = meta['C']
    NPC = meta['NPC']
    NPC1 = meta['NPC1']
    NT = meta['NT']
    FP = meta['FP']
    C_LO = meta['C_LO']
    K_lo = meta['K_lo']
    K_hi = meta['K_hi']
    groups = meta['groups']
    lo_coffs = meta['lo_coffs']
    hi_coffs = meta['hi_coffs']
    SUMW_LO = meta['SUMW_LO']
    SUMW_HI = meta['SUMW_HI']
    NF = FP // P
    MBLK = 512
    n_mblk = (NPC + MBLK - 1) // MBLK
    NLO = C_LO * NPC1
    NHI = (C - C_LO) * NPC1
    GK_LO = max(g[2] for g in groups)
    GK_HI = max(g[3] for g in groups)

    nc = bacc.Bacc("TRN2", target_bir_lowering=False, debug=False,
                   num_devices=C, num_swdge_queues=4)

    xt_d = nc.dram_tensor("xt", [FP, NPC], mm_dtype, kind="ExternalInput")
    w1_d = nc.dram_tensor("w1", [FP, H], mm_dtype, kind="ExternalInput")
    w2_d = nc.dram_tensor("w2", [2 * H, H], F32, kind="ExternalInput")
    b1_d = nc.dram_tensor("b1r", [P, H], F32, kind="ExternalInput")
    b2_d = nc.dram_tensor("b2r", [P, H], F32, kind="ExternalInput")
    fcw_d = nc.dram_tensor("fcwr", [P, H], F32, kind="ExternalInput")
    fcb_d = nc.dram_tensor("fcbr", [P, 1], F32, kind="ExternalInput")
    ilo_d = nc.dram_tensor("itbl_lo", [P, SUMW_LO], I16, kind="ExternalInput")
    ihi_d = nc.dram_tensor("itbl_hi", [P, SUMW_HI], I16, kind="ExternalInput")
    x1lo_d = nc.dram_tensor("x1w_lo", [P, 8 * NT], I16, kind="ExternalInput")
    x1hi_d = nc.dram_tensor("x1w_hi", [P, 8 * NT], I16, kind="ExternalInput")
    degt_d = nc.dram_tensor("degt", [P, NT], F32, kind="ExternalInput")
    keep_d = nc.dram_tensor("keep", [P, NT], F32, kind="ExternalInput")
    mneg_d = nc.dram_tensor("mneg", [P, NT], F32, kind="ExternalInput")
    y_d = nc.dram_tensor("y", [P, NT], F32, kind="ExternalOutput")
    if debug_taps:
        z1dbg_d = nc.dram_tensor("z1dbg", [NPC1, H], F32, kind="ExternalOutput")
        udbg_d = nc.dram_tensor("udbg", [NPC1, H], F32, kind="ExternalOutput")
        z2dbg_d = nc.dram_tensor("z2dbg", [NPC1, H], F32, kind="ExternalOutput")
        vdbg_d = nc.dram_tensor("vdbg", [P, NT], F32, kind="ExternalOutput")
        gdbg_d = nc.dram_tensor("gdbg", [P, groups[0][2] * H], F32,
                                kind="ExternalOutput")

    rg = [list(range(C))]

    with tile.TileContext(nc) as tc:
        with tc.tile_pool(name="const", bufs=1) as cpool, \
             tc.tile_pool(name="xin", bufs=4) as xpool, \
             tc.tile_pool(name="work", bufs=2) as wpool, \
             tc.tile_pool(name="gath", bufs=3) as gpool, \
             tc.tile_pool(name="big", bufs=1) as bpool, \
             tc.tile_pool(name="ps", bufs=4, space="PSUM") as pspool, \
             tc.tile_pool(name="psz", bufs=2, space="PSUM") as pszpool, \
             tc.tile_pool(name="dram", bufs=1, space="DRAM") as dpool:

            z1b = dpool.tile([NPC1, H], F32)
            ub = dpool.tile([NPC1, H], F32)
            z2b = dpool.tile([NPC1, H], F32)
            ash = "Shared" if C > 4 else "Local"
            # z1/z2: AllGather direct to Local; u: AllGather to Shared then
            # bulk-copy to Local (A/B comparison of gather-table residency).
            z1g = nc.dram_tensor("z1g_loc", [C * NPC1, H], F32,
                                 kind="Internal", addr_space="Local").ap()
            ug_sh = nc.dram_tensor("ug_sh", [C * NPC1, H], F32,
                                   kind="Internal", addr_space=ash).ap()
            ug = nc.dram_tensor("ug_loc", [C * NPC1, H], F32,
                                kind="Internal", addr_space="Local").ap()
            z2g = nc.dram_tensor("z2g_loc", [C * NPC1, H], F32,
                                 kind="Internal", addr_space="Local").ap()
            sj_in = dpool.tile([1, 1], F32)
            sj_out = nc.dram_tensor("sj_sh", [1, 1], F32,
                                    kind="Internal", addr_space=ash).ap()

            ident = cpool.tile([P, P], F32)
            make_identity(nc, ident[:])
            w1_sb = cpool.tile([P, NF * H], mm_dtype)
            nc.sync.dma_start(
                w1_sb[:], w1_d.ap().rearrange("(a p) h -> p a h", p=P))
            w2_sb = cpool.tile([P, H], F32)
            nc.sync.dma_start(w2_sb[:], w2_d.ap())
            b1_sb = cpool.tile([P, H], F32)
            nc.sync.dma_start(b1_sb[:], b1_d.ap())
            b2_sb = cpool.tile([P, H], F32)
            nc.sync.dma_start(b2_sb[:], b2_d.ap())
            fcw_sb = cpool.tile([P, H], F32)
            nc.sync.dma_start(fcw_sb[:], fcw_d.ap())
            fcb_sb = cpool.tile([P, 1], F32)
            nc.sync.dma_start(fcb_sb[:], fcb_d.ap())
            ilo_sb = cpool.tile([P, SUMW_LO], I16)
            nc.sync.dma_start(ilo_sb[:], ilo_d.ap())
            ihi_sb = cpool.tile([P, SUMW_HI], I16)
            nc.sync.dma_start(ihi_sb[:], ihi_d.ap())
            x1lo_sb = cpool.tile([P, 8 * NT], I16)
            nc.sync.dma_start(x1lo_sb[:], x1lo_d.ap())
            x1hi_sb = cpool.tile([P, 8 * NT], I16)
            nc.sync.dma_start(x1hi_sb[:], x1hi_d.ap())
            degt_sb = cpool.tile([P, NT], F32)
            nc.sync.dma_start(degt_sb[:], degt_d.ap())
            keep_sb = cpool.tile([P, NT], F32)
            nc.sync.dma_start(keep_sb[:], keep_d.ap())
            mneg_sb = cpool.tile([P, NT], F32)
            nc.sync.dma_start(mneg_sb[:], mneg_d.ap())

            dinv_sb = cpool.tile([P, NT], F32)
            nc.vector.reciprocal(dinv_sb[:], degt_sb[:])
            nc.scalar.activation(dinv_sb[:], dinv_sb[:], ACTF.Sqrt)

            zrow_sb = cpool.tile([1, H], F32)
            nc.vector.memset(zrow_sb[:], 0.0)
            nc.sync.dma_start(z1b[NPC:NPC + 1, :], zrow_sb[:])
            nc.sync.dma_start(ub[NPC:NPC + 1, :], zrow_sb[:])
            nc.sync.dma_start(z2b[NPC:NPC + 1, :], zrow_sb[:])

            ones_sb = cpool.tile([P, 1], F32)
            nc.vector.memset(ones_sb[:], 1.0)
            ones_row = cpool.tile([1, P], F32)
            nc.vector.memset(ones_row[:], 1.0)
            neg48_sb = cpool.tile([P, 1], F32)
            nc.vector.memset(neg48_sb[:], -SOFTMAX_SHIFT)

            u_loc = bpool.tile([P, NT * 2 * H], F32)   # [u | x1u] interleaved

            def elu_into(dst_ap, src_ap, tmp_pool, fd):
                mn = tmp_pool.tile([P, fd], F32, tag="elu_mn")
                ex = tmp_pool.tile([P, fd], F32, tag="elu_ex")
                nc.vector.tensor_scalar(out=mn[:], in0=src_ap, scalar1=0.0,
                                        scalar2=None, op0=ALU.min)
                nc.scalar.activation(ex[:], mn[:], ACTF.Exp)
                nc.vector.tensor_scalar(out=mn[:], in0=src_ap, scalar1=0.0,
                                        scalar2=None, op0=ALU.max)
                nc.vector.tensor_scalar(out=ex[:], in0=ex[:], scalar1=-1.0,
                                        scalar2=None, op0=ALU.add)
                nc.vector.tensor_tensor(out=dst_ap, in0=mn[:], in1=ex[:],
                                        op=ALU.add)

            def agg_layer(src_g, out_cb, tap=False):
                qn = [0]
                for gi, (a, b, sklo, skhi) in enumerate(groups):
                    glo = gpool.tile([P, GK_LO * H], F32, tag="glo")
                    ghi = gpool.tile([P, GK_HI * H], F32, tag="ghi")
                    nlo_idx = sklo * P
                    nhi_idx = skhi * P
                    nc.gpsimd.dma_gather(
                        out_ap=glo[:, :sklo * H].rearrange(
                            "p (c h) -> p c h", h=H),
                        in_ap=src_g[0:NLO, :],
                        idxs_ap=ilo_sb[:, lo_coffs[gi]:lo_coffs[gi + 1]],
                        num_idxs=nlo_idx, num_idxs_reg=nlo_idx, elem_size=H,
                        single_packet=False, queue_num=(2 * gi) % 4)
                    nc.gpsimd.dma_gather(
                        out_ap=ghi[:, :skhi * H].rearrange(
                            "p (c h) -> p c h", h=H),
                        in_ap=src_g[NLO:NLO + NHI, :],
                        idxs_ap=ihi_sb[:, hi_coffs[gi]:hi_coffs[gi + 1]],
                        num_idxs=nhi_idx, num_idxs_reg=nhi_idx, elem_size=H,
                        single_packet=False, queue_num=(2 * gi + 1) % 4)
                    if tap and gi == 0:
                        nc.sync.dma_start(gdbg_d.ap()[:], glo[:, :sklo * H])
                    ko = 0
                    kho = 0
                    for t in range(a, b):
                        klo_t = int(K_lo[t])
                        khi_t = int(K_hi[t])
                        ra = wpool.tile([P, H], F32, tag="ra")
                        nc.vector.tensor_reduce(
                            out=ra[:],
                            in_=glo[:, ko * H:(ko + klo_t) * H].rearrange(
                                "p (k h) -> p h k", k=klo_t),
                            op=ALU.add, axis=AX.X)
                        rb = wpool.tile([P, H], F32, tag="rb")
                        nc.vector.tensor_reduce(
                            out=rb[:],
                            in_=ghi[:, kho * H:(kho + khi_t) * H].rearrange(
                                "p (k h) -> p h k", k=khi_t),
                            op=ALU.add, axis=AX.X)
                        nc.vector.tensor_tensor(out=ra[:], in0=ra[:],
                                                in1=rb[:], op=ALU.add)
                        agg = wpool.tile([P, H], F32, tag="agg")
                        nc.vector.tensor_scalar(
                            out=agg[:], in0=ra[:],
                            scalar1=dinv_sb[:, t:t + 1], scalar2=None,
                            op0=ALU.mult)
                        out_cb(t, agg)
                        ko += klo_t
                        kho += khi_t

            # ================= P1: z1 = x @ W1 ===========================
            sc = nc.enter_named_scope("p1", False)[0]
            for mp in range(0, n_mblk, 2):
                nmb = min(2, n_mblk - mp)
                w0 = min(MBLK * nmb, NPC - mp * MBLK)
                zt_list = [pszpool.tile([H, MBLK], F32, tag=f"zt{j}",
                                        name=f"ztp{j}")
                           for j in range(nmb)]
                for f in range(NF):
                    xt_sb = xpool.tile([P, 2 * MBLK], mm_dtype, tag="xt")
                    nc.sync.dma_start(
                        xt_sb[:, :w0],
                        xt_d.ap()[f * P:(f + 1) * P,
                                  mp * MBLK:mp * MBLK + w0])
                    for j in range(nmb):
                        mw = min(MBLK, NPC - (mp + j) * MBLK)
                        nc.tensor.matmul(
                            zt_list[j][:, :mw],
                            lhsT=w1_sb[:, f * H:(f + 1) * H],
                            rhs=xt_sb[:, j * MBLK:j * MBLK + mw],
                            start=(f == 0), stop=(f == NF - 1))
                for j in range(nmb):
                    m = mp + j
                    mw = min(MBLK, NPC - m * MBLK)
                    zt_sb = wpool.tile([H, MBLK], F32, tag="zt_sb")
                    nc.vector.tensor_copy(zt_sb[:, :mw], zt_list[j][:, :mw])
                    for k in range(mw // P):
                        t = m * (MBLK // P) + k
                        tr_ps = pspool.tile([P, H], F32, tag="pss")
                        nc.tensor.transpose(
                            tr_ps[:], zt_sb[:, k * P:(k + 1) * P],
                            ident[:H, :H])
                        z1p_sb = wpool.tile([P, H], F32, tag="z1p")
                        nc.vector.tensor_scalar(
                            out=z1p_sb[:], in0=tr_ps[:],
                            scalar1=dinv_sb[:, t:t + 1], scalar2=None,
                            op0=ALU.mult)
                        nc.sync.dma_start(z1b[t * P:(t + 1) * P, :],
                                          z1p_sb[:])

            nc.leave_named_scope("p1", sc, False)

            # ================= P2: AllGather z1' (direct to Local) =======
            sc = nc.enter_named_scope("ag1", False)[0]
            nc.gpsimd.collective_compute(
                "AllGather", ALU.bypass, replica_groups=rg,
                ins=[z1b.opt()], outs=[z1g[:, :]])
            nc.leave_named_scope("ag1", sc, False)

            # ================= P3: conv1 agg + elu =======================
            sc = nc.enter_named_scope("p3", False)[0]
            def p3_out(t, agg):
                if not skip_b1:
                    nc.vector.tensor_tensor(out=agg[:], in0=agg[:],
                                            in1=b1_sb[:], op=ALU.add)
                udst = u_loc[:, t * 2 * H: t * 2 * H + H]
                elu_into(udst, agg[:], wpool, H)
                nc.sync.dma_start(ub[t * P:(t + 1) * P, :], udst)
            agg_layer(z1g, p3_out, tap=debug_taps)
            nc.leave_named_scope("p3", sc, False)

            # ================= P4: AllGather u (Shared, then copy) =======
            sc = nc.enter_named_scope("ag2", False)[0]
            nc.gpsimd.collective_compute(
                "AllGather", ALU.bypass, replica_groups=rg,
                ins=[ub.opt()], outs=[ug_sh[:, :]])
            nc.sync.dma_start(ug[0:NLO, :], ug_sh[0:NLO, :])
            nc.scalar.dma_start(ug[NLO:NLO + NHI, :], ug_sh[NLO:NLO + NHI, :])
            nc.leave_named_scope("ag2", sc, False)

            # ================= P5: x1 gather + z2 ========================
            sc = nc.enter_named_scope("p5", False)[0]
            xga = bpool.tile([P, NT * H], F32)
            xgb = bpool.tile([P, NT * H], F32)
            nidx = NT * P
            nc.gpsimd.dma_gather(
                out_ap=xga[:].rearrange("p (c h) -> p c h", h=H),
                in_ap=ug[0:NLO, :], idxs_ap=x1lo_sb[:, :],
                num_idxs=nidx, num_idxs_reg=nidx, elem_size=H,
                single_packet=False, queue_num=0)
            nc.gpsimd.dma_gather(
                out_ap=xgb[:].rearrange("p (c h) -> p c h", h=H),
                in_ap=ug[NLO:NLO + NHI, :], idxs_ap=x1hi_sb[:, :],
                num_idxs=nidx, num_idxs_reg=nidx, elem_size=H,
                single_packet=False, queue_num=1)
            nc.vector.tensor_tensor(
                out=u_loc[:].rearrange("p (t h) -> p t h", t=2 * NT)[:, 1::2, :],
                in0=xga[:].rearrange("p (t h) -> p t h", t=NT),
                in1=xgb[:].rearrange("p (t h) -> p t h", t=NT),
                op=ALU.add)
            for t in range(NT):
                hT_ps = pspool.tile([P, P], F32, tag="pss")
                nc.tensor.transpose(
                    hT_ps[:], u_loc[:, t * 2 * H:(t + 1) * 2 * H], ident[:])
                hT_sb = wpool.tile([P, P], F32, tag="hT_sb")
                nc.vector.tensor_copy(hT_sb[:], hT_ps[:])
                z2_ps = pspool.tile([P, H], F32, tag="pss")
                nc.tensor.matmul(z2_ps[:], lhsT=hT_sb[:], rhs=w2_sb[:],
                                 start=True, stop=True)
                z2p_sb = wpool.tile([P, H], F32, tag="z2p")
                nc.vector.tensor_scalar(
                    out=z2p_sb[:], in0=z2_ps[:], scalar1=dinv_sb[:, t:t + 1],
                    scalar2=None, op0=ALU.mult)
                nc.sync.dma_start(z2b[t * P:(t + 1) * P, :], z2p_sb[:])

            nc.leave_named_scope("p5", sc, False)

            # ================= P6: AllGather z2' (direct to Local) =======
            sc = nc.enter_named_scope("ag3", False)[0]
            nc.gpsimd.collective_compute(
                "AllGather", ALU.bypass, replica_groups=rg,
                ins=[z2b.opt()], outs=[z2g[:, :]])
            nc.leave_named_scope("ag3", sc, False)

            # ================= P7: conv2 agg + head ======================
            sc = nc.enter_named_scope("p7", False)[0]
            vbuf = bpool.tile([P, NT], F32)

            def p7_out(t, agg):
                if not skip_b2:
                    nc.vector.tensor_tensor(out=agg[:], in0=agg[:],
                                            in1=b2_sb[:], op=ALU.add)
                e2 = wpool.tile([P, H], F32, tag="e2")
                elu_into(e2[:], agg[:], wpool, H)
                nc.vector.tensor_tensor(out=e2[:], in0=e2[:], in1=fcw_sb[:],
                                        op=ALU.mult)
                nc.vector.tensor_reduce(out=vbuf[:, t:t + 1], in_=e2[:],
                                        op=ALU.add, axis=AX.X)
            agg_layer(z2g, p7_out)
            nc.leave_named_scope("p7", sc, False)

            sc = nc.enter_named_scope("head", False)[0]
            nc.vector.tensor_tensor(out=vbuf[:], in0=vbuf[:], in1=keep_sb[:],
                                    op=ALU.mult)
            nc.vector.tensor_tensor(out=vbuf[:], in0=vbuf[:], in1=mneg_sb[:],
                                    op=ALU.add)
            es = bpool.tile([P, NT], F32)
            acc = wpool.tile([P, 1], F32, tag="acc")
            nc.scalar.activation(es[:], vbuf[:], ACTF.Exp,
                                 bias=neg48_sb[:], scale=1.0,
                                 accum_out=acc[:])
            s_ps = pspool.tile([1, 1], F32, tag="pss")
            nc.tensor.matmul(s_ps[:], lhsT=acc[:], rhs=ones_sb[:],
                             start=True, stop=True)
            s_sb = wpool.tile([1, 1], F32, tag="s_sb")
            nc.vector.tensor_copy(s_sb[:], s_ps[:])
            nc.sync.dma_start(sj_in[:], s_sb[:])
            nc.gpsimd.collective_compute(
                "AllReduce", ALU.add, replica_groups=rg,
                ins=[sj_in.opt()], outs=[sj_out[:, :]])
            s2_sb = wpool.tile([1, 1], F32, tag="s2_sb")
            nc.sync.dma_start(s2_sb[:], sj_out[:, :])
            lnS = wpool.tile([1, 1], F32, tag="lnS")
            nc.scalar.activation(lnS[:], s2_sb[:], ACTF.Ln)
            b_ps = pspool.tile([P, 1], F32, tag="pss")
            nc.tensor.matmul(b_ps[:], lhsT=ones_row[:], rhs=lnS[:],
                             start=True, stop=True)
            bias_sb = wpool.tile([P, 1], F32, tag="bias_sb")
            nc.vector.tensor_scalar(out=bias_sb[:], in0=b_ps[:],
                                    scalar1=-1.0, scalar2=-SOFTMAX_SHIFT,
                                    op0=ALU.mult, op1=ALU.add)
            y_sb = bpool.tile([P, NT], F32)
            nc.vector.tensor_tensor(out=y_sb[:], in0=vbuf[:],
                                    in1=bias_sb[:].to_broadcast([P, NT]),
                                    op=ALU.add)
            nc.sync.dma_start(y_d.ap()[:], y_sb[:])
            nc.leave_named_scope("head", sc, False)
            if debug_taps:
                nc.sync.dma_start(z1dbg_d.ap()[:], z1b[:])
                nc.sync.dma_start(udbg_d.ap()[:], ub[:])
                nc.sync.dma_start(z2dbg_d.ap()[:], z2b[:])
                nc.sync.dma_start(vdbg_d.ap()[:], vbuf[:])

    nc.compile()
    return nc


# ---------------------------------------------------------------------------
# Full flow
# ---------------------------------------------------------------------------

def run(x, edge_index, all_edge_index, s_mapping_index, e_mask,
        conv1_w, conv1_b, conv2_w, conv2_b, fc_w, fc_b,
        C=8, mm_dtype=F32, trace=False, nc_cache=None, debug_taps=False,
        **rbk_kwargs):
    tabs, meta = host_prep(
        x, edge_index, all_edge_index, s_mapping_index, e_mask, C)
    w1, w2, b1r, b2r, fcwr, fcbr = host_prep_weights(
        conv1_w, conv1_b, conv2_w, conv2_b, fc_w, fc_b, meta)
    fcb_val = np.float32(np.asarray(fc_b).reshape(-1)[0])
    for c in range(C):
        tabs['mneg'][c] = (tabs['mneg'][c]
                           + fcb_val * tabs['keep'][c]).astype(np.float32)
    skip_b1 = bool(np.all(np.asarray(conv1_b) == 0))
    skip_b2 = bool(np.all(np.asarray(conv2_b) == 0))

    if nc_cache is not None and 'nc' in nc_cache:
        nc = nc_cache['nc']
    else:
        nc = build_kernel(meta, mm_dtype=mm_dtype, debug_taps=debug_taps,
                          skip_b1=skip_b1, skip_b2=skip_b2)
        if nc_cache is not None:
            nc_cache['nc'] = nc

    if mm_dtype == mybir.dt.bfloat16:
        import ml_dtypes
        w1 = w1.astype(ml_dtypes.bfloat16)
        tabs['xts'] = [xt.astype(ml_dtypes.bfloat16) for xt in tabs['xts']]

    in_maps = []
    for c in range(C):
        in_maps.append(dict(
            xt=tabs['xts'][c], w1=w1, w2=w2, b1r=b1r, b2r=b2r, fcwr=fcwr,
            fcbr=fcbr, itbl_lo=tabs['itbl_lo'][c], itbl_hi=tabs['itbl_hi'][c],
            x1w_lo=tabs['x1w_lo'][c], x1w_hi=tabs['x1w_hi'][c],
            degt=tabs['degt'][c], keep=tabs['keep'][c], mneg=tabs['mneg'][c]))
    res = bass_utils.run_bass_kernel_spmd(
        nc, in_maps, core_ids=list(range(C)), trace=trace, **rbk_kwargs)

    N = meta['N']
    n_per = meta['n_per']
    out = np.empty((N, 1), dtype=np.float32)
    for c in range(C):
        yc = res.results[c]['y']
        flat = yc.T.reshape(-1)
        out[meta['lperm'][c], 0] = flat[:n_per]
    return out, res, meta


# ---------------------------------------------------------------------------
# Harness entry point
# ---------------------------------------------------------------------------

_NC_CACHE = {}


def kernel(**inputs):
    """Full (unsharded) inputs -> full [N, 1] float32 output."""
    out, _res, _meta = run(
        x=np.asarray(inputs['x'], dtype=np.float32),
        edge_index=np.asarray(inputs['edge_index']),
        all_edge_index=np.asarray(inputs['all_edge_index']),
        s_mapping_index=np.asarray(inputs['s_mapping_index']),
        e_mask=np.asarray(inputs['e_mask']),
        conv1_w=np.asarray(inputs['conv1_w'], dtype=np.float32),
        conv1_b=np.asarray(inputs['conv1_b'], dtype=np.float32),
        conv2_w=np.asarray(inputs['conv2_w'], dtype=np.float32),
        conv2_b=np.asarray(inputs['conv2_b'], dtype=np.float32),
        fc_w=np.asarray(inputs['fc_w'], dtype=np.float32),
        fc_b=np.asarray(inputs['fc_b'], dtype=np.float32),
        C=8, mm_dtype=mybir.dt.bfloat16, trace=False, nc_cache=_NC_CACHE)
    return out

